# revision 7
# baseline (speedup 1.0000x reference)
"""Trainium2 Bass kernel for nn_FAA (Fourier-argmax alignment).

Per sample (7x7 image): rfft2 magnitudes -> argmax over 27 frequency bins
(weighted by rho) -> rotation angle theta -> bilinear grid-sample rotation.

Device pipeline (2 launches, 8 cores, batch sharded):
  Launch 1 (decision): FFT as matmul (fp16 hi/lo, fp32-accurate), square,
    |F|^2*rho^2 per bin, argmax via mantissa-packed segmented max-reduce.
  Host glue: bin -> unique-angle class, class-sort layout (counting sort).
  Launch 2 (apply): per-class rotation matrices as dense matmuls over the
    class-sorted stream (fp16 hi/lo, fp32-accurate).
"""

import os
import sys

import numpy as np

sys.path.insert(0, "/opt/trn_rl_repo")

import concourse.bacc as bacc
import concourse.tile as tile
from concourse import mybir
from concourse.bass_utils import run_bass_kernel_spmd

F32 = mybir.dt.float32
F16 = mybir.dt.float16
U32 = mybir.dt.uint32

H = W = 7
HW = 49
B = 524288
N_CORES = 8
PER = B // N_CORES  # 65536
GRP = 1024  # samples per launch-1 group (2 subtiles of 512)
N_GRP = PER // GRP  # 64
STAGE_GROUPS = 8  # groups per Mx staging DMA
TILE = 512  # samples per launch-2 matmul tile

LAST_EXEC_NS = {"decision": None, "apply": None}
_TRACE = os.environ.get("FAA_TRACE", "0") == "1"

# ---------------------------------------------------------------------------
# constants (pure math, replicates the reference module exactly)
# ---------------------------------------------------------------------------


def _freq_constants():
    h_shift = np.roll(np.arange(H) - H // 2, H // 2)
    w_shift = np.concatenate([np.arange(W // 2 + 1)[: W // 2], np.array([-(W // 2)])])
    y, xg = np.meshgrid(h_shift, w_shift, indexing="ij")
    rho = np.sqrt((xg**2 + y**2).astype(np.float32))
    theta = np.arctan2(y.astype(np.float32), xg.astype(np.float32))
    theta = (theta + 2.0 * np.pi) % (2.0 * np.pi)
    valid = np.flatnonzero((rho > 1e-8).ravel())
    return valid, theta.ravel()[valid].astype(np.float32), rho.ravel()[valid].astype(
        np.float32
    )


VALID_IDX, VALID_THETAS, VALID_RHOS = _freq_constants()
NBINS = len(VALID_IDX)  # 27


def _fourier_mats():
    C = np.zeros((NBINS, HW), dtype=np.float64)
    S = np.zeros((NBINS, HW), dtype=np.float64)
    for j, flat in enumerate(VALID_IDX):
        u, v = divmod(int(flat), W // 2 + 1)
        for h in range(H):
            for w in range(W):
                ang = 2.0 * np.pi * (u * h + v * w) / 7.0
                C[j, h * W + w] = np.cos(ang) / 7.0
                S[j, h * W + w] = -np.sin(ang) / 7.0
    return C.astype(np.float32), S.astype(np.float32)


CMAT, SMAT = _fourier_mats()

_ys = np.linspace(-1.0, 1.0, H, dtype=np.float32)
_xs = np.linspace(-1.0, 1.0, W, dtype=np.float32)
GY, GX = np.meshgrid(_ys, _xs, indexing="ij")[0], np.meshgrid(_ys, _xs, indexing="ij")[
    1
]


def _rotation_matrix(theta):
    c = np.float32(np.cos(theta))
    s = np.float32(np.sin(theta))
    sx = GX * c - GY * s
    sy = GX * s + GY * c
    ix = np.clip((sx + np.float32(1)) * np.float32(0.5) * (W - 1), 0.0, W - 1).astype(
        np.float32
    )
    iy = np.clip((sy + np.float32(1)) * np.float32(0.5) * (H - 1), 0.0, H - 1).astype(
        np.float32
    )
    ix0f = np.floor(ix)
    iy0f = np.floor(iy)
    wx = (ix - ix0f).astype(np.float32)
    wy = (iy - iy0f).astype(np.float32)
    ix0 = ix0f.astype(np.int64)
    iy0 = iy0f.astype(np.int64)
    ix1 = np.minimum(ix0 + 1, W - 1)
    iy1 = np.minimum(iy0 + 1, H - 1)
    A = np.zeros((HW, HW), dtype=np.float32)
    for r in range(H):
        for cc in range(W):
            p = r * W + cc
            A[p, iy0[r, cc] * W + ix0[r, cc]] += (1 - wy[r, cc]) * (1 - wx[r, cc])
            A[p, iy0[r, cc] * W + ix1[r, cc]] += (1 - wy[r, cc]) * wx[r, cc]
            A[p, iy1[r, cc] * W + ix0[r, cc]] += wy[r, cc] * (1 - wx[r, cc])
            A[p, iy1[r, cc] * W + ix1[r, cc]] += wy[r, cc] * wx[r, cc]
    return A


UNIQ_THETAS, BIN2UID = np.unique(VALID_THETAS, return_inverse=True)
NU = len(UNIQ_THETAS)  # 20
AMATS = np.stack([_rotation_matrix(t) for t in UNIQ_THETAS])  # [NU, 49, 49]


def _split16(a):
    hi = a.astype(np.float16)
    lo = (a.astype(np.float32) - hi.astype(np.float32)).astype(np.float16)
    return hi, lo


def _build_l1_consts():
    # CS64 [49, 64]: col 2j -> rho_j*C_j ; col 2j+1 -> rho_j*S_j (interleaved
    # so Re^2+Im^2 is a segmented pair reduce after transpose)
    cs = np.zeros((HW, 64), dtype=np.float32)
    cs[:, 0 : 2 * NBINS : 2] = (CMAT * VALID_RHOS[:, None]).T
    cs[:, 1 : 2 * NBINS : 2] = (SMAT * VALID_RHOS[:, None]).T
    cs_hi, cs_lo = _split16(cs)
    # term A lhsT [98, 64] = [Clo ; Chi] (matches rhs [Xhi ; Xlo])
    lhsA = np.concatenate([cs_lo, cs_hi], axis=0)
    lhsB = cs_hi  # [49, 64], rhs = Xhi rows
    ident = np.eye(128, dtype=np.float32)
    iota = np.broadcast_to(
        np.tile(np.arange(32, dtype=np.uint32), 8), (128, 8 * 32)
    ).copy()
    return lhsA, lhsB, ident, iota


L1_LHSA, L1_LHSB, L1_IDENT, L1_IOTA = _build_l1_consts()


def _build_l2_consts():
    # per class u: lhsT = A_u^T (K=in-pixel, M=out-pixel)
    at = AMATS.transpose(0, 2, 1).astype(np.float32)  # [NU, 49, 49] lhsT
    hi, lo = _split16(at)
    lhsA = np.concatenate([lo, hi], axis=1).reshape(NU, 98, HW)  # [Alo; Ahi] stacked K
    lhsA = np.concatenate(list(lhsA), axis=1)  # [98, NU*49]
    lhsB = np.concatenate(list(hi), axis=1)  # [49, NU*49]
    return np.ascontiguousarray(lhsA), np.ascontiguousarray(lhsB)


L2_LHSA, L2_LHSB = _build_l2_consts()

# ---------------------------------------------------------------------------
# launch 1: decision kernel
# ---------------------------------------------------------------------------


def build_decision_kernel():
    nc = bacc.Bacc("TRN2", target_bir_lowering=False, debug=False, num_devices=N_CORES)
    xt2 = nc.dram_tensor("xt2", [98, PER], F16, kind="ExternalInput")
    lhsa = nc.dram_tensor("lhsa", [98, 64], F16, kind="ExternalInput")
    lhsb = nc.dram_tensor("lhsb", [49, 64], F16, kind="ExternalInput")
    ident = nc.dram_tensor("ident", [128, 128], F32, kind="ExternalInput")
    iota = nc.dram_tensor("iota", [128, 8 * 32], U32, kind="ExternalInput")
    mx = nc.dram_tensor(
        "mx", [N_GRP // STAGE_GROUPS, 128, STAGE_GROUPS * 8], F32, kind="ExternalOutput"
    )

    with tile.TileContext(nc) as tc:
        with (
            tc.tile_pool(name="const", bufs=1) as cpool,
            tc.tile_pool(name="xin", bufs=4) as xpool,
            tc.tile_pool(name="sq", bufs=3) as sqpool,
            tc.tile_pool(name="w", bufs=3) as wpool,
            tc.tile_pool(name="stage", bufs=3) as stpool,
            tc.tile_pool(name="psF", bufs=2, space="PSUM") as psF,
            tc.tile_pool(name="psT", bufs=2, space="PSUM") as psT,
        ):
            c_lhsa = cpool.tile([98, 64], F16)
            c_lhsb = cpool.tile([49, 64], F16)
            c_id = cpool.tile([128, 128], F32)
            c_iota = cpool.tile([128, 8 * 32], U32)
            nc.sync.dma_start(out=c_lhsa, in_=lhsa.ap())
            nc.sync.dma_start(out=c_lhsb, in_=lhsb.ap())
            nc.sync.dma_start(out=c_id, in_=ident.ap())
            nc.sync.dma_start(out=c_iota, in_=iota.ap())

            xt2_ap = xt2.ap()
            for gs in range(N_GRP // STAGE_GROUPS):
                stg = stpool.tile([128, STAGE_GROUPS * 8], F32, tag="stage")
                for gi in range(STAGE_GROUPS):
                    g = gs * STAGE_GROUPS + gi
                    base = g * GRP
                    # load 2 subtiles [98, 512] fp16
                    xa = xpool.tile([98, TILE], F16, tag="xa")
                    xb = xpool.tile([98, TILE], F16, tag="xb")
                    nc.sync.dma_start(out=xa, in_=xt2_ap[:, base : base + TILE])
                    nc.sync.dma_start(
                        out=xb, in_=xt2_ap[:, base + TILE : base + 2 * TILE]
                    )
                    # FFT -> psum F [128, 512] (2 subtiles x 64 comps)
                    f4 = psF.tile([128, TILE], F32, tag="f4")
                    nc.tensor.matmul(f4[0:64, :], c_lhsa, xa, start=True, stop=False)
                    nc.tensor.matmul(
                        f4[0:64, :], c_lhsb, xa[0:49, :], start=False, stop=True
                    )
                    nc.tensor.matmul(
                        f4[64:128, :],
                        c_lhsa,
                        xb,
                        start=True,
                        stop=False,
                        tile_position=(0, 64),
                    )
                    nc.tensor.matmul(
                        f4[64:128, :],
                        c_lhsb,
                        xb[0:49, :],
                        start=False,
                        stop=True,
                        tile_position=(0, 64),
                    )
                    # square (ACT) -> sbuf
                    fsq = sqpool.tile([128, TILE], F32, tag="fsq")
                    nc.scalar.activation(
                        out=fsq, in_=f4, func=mybir.ActivationFunctionType.Square
                    )
                    # PE transpose 4 blocks -> psum FsqT [128, 512]
                    ft = psT.tile([128, TILE], F32, tag="ft")
                    for b in range(4):
                        nc.tensor.transpose(
                            ft[:, 128 * b : 128 * (b + 1)],
                            fsq[:, 128 * b : 128 * (b + 1)],
                            c_id,
                        )
                    # W = Re^2 + Im^2 : segmented pair reduce [128, 8, 32]
                    wt = wpool.tile([128, 8, 32], F32, tag="wt")
                    nc.vector.tensor_reduce(
                        out=wt,
                        in_=ft.rearrange("p (s b t) -> p s b t", s=8, b=32),
                        axis=mybir.AxisListType.X,
                        op=mybir.AluOpType.add,
                    )
                    # pack bin id into low 5 mantissa bits
                    wm = wpool.tile([128, 8, 32], F32, tag="wm")
                    nc.vector.tensor_scalar(
                        out=wm.bitcast(U32),
                        in0=wt.bitcast(U32),
                        scalar1=0xFFFFFFE0,
                        scalar2=None,
                        op0=mybir.AluOpType.bitwise_and,
                    )
                    wp = wpool.tile([128, 8, 32], F32, tag="wp")
                    nc.vector.tensor_tensor(
                        out=wp.bitcast(U32),
                        in0=wm.bitcast(U32),
                        in1=c_iota.rearrange("p (s c) -> p s c", c=32),
                        op=mybir.AluOpType.bitwise_or,
                    )
                    # segmented max over 27 bins -> [128, 8]
                    nc.vector.tensor_reduce(
                        out=stg[:, gi * 8 : (gi + 1) * 8],
                        in_=wp,
                        axis=mybir.AxisListType.X,
                        op=mybir.AluOpType.max,
                    )
                nc.sync.dma_start(out=mx.ap()[gs], in_=stg)
    nc.compile()
    return nc


def decode_mx(mx_arr):
    """mx [N_GRP//SG, 128, SG*8] f32 -> k [PER] int (bin index)."""
    bits = np.minimum(mx_arr.view(np.uint32) & np.uint32(31), NBINS - 1)
    # [gs, i, gi*8 + j] ; j = 2*b + s ; sample = (gs*SG+gi)*1024 + s*512 + b*128 + i
    k = np.empty(PER, dtype=np.int64)
    gs_, i_, c_ = np.indices(bits.shape)
    gi_ = c_ // 8
    j_ = c_ % 8
    b_ = j_ // 2
    s_ = j_ % 2
    sample = (gs_ * STAGE_GROUPS + gi_) * GRP + s_ * TILE + b_ * 128 + i_
    k[sample.ravel()] = bits.ravel()
    return k


# ---------------------------------------------------------------------------
# launch 2: apply kernel (schedule depends on per-class capacities)
# ---------------------------------------------------------------------------


def build_apply_kernel(caps):
    cap_total = int(np.sum(caps))
    nc = bacc.Bacc("TRN2", target_bir_lowering=False, debug=False, num_devices=N_CORES)
    xst2 = nc.dram_tensor("xst2", [98, cap_total], F16, kind="ExternalInput")
    lhsa = nc.dram_tensor("lhsa", [98, NU * HW], F16, kind="ExternalInput")
    lhsb = nc.dram_tensor("lhsb", [49, NU * HW], F16, kind="ExternalInput")
    yst = nc.dram_tensor("yst", [49, cap_total], F32, kind="ExternalOutput")

    # tile schedule: (class u, slot base)
    sched = []
    off = 0
    for u in range(NU):
        for t in range(int(caps[u]) // TILE):
            sched.append((u, off + t * TILE))
        off += int(caps[u])

    with tile.TileContext(nc) as tc:
        with (
            tc.tile_pool(name="const", bufs=1) as cpool,
            tc.tile_pool(name="xin", bufs=4) as xpool,
            tc.tile_pool(name="yout", bufs=4) as ypool,
            tc.tile_pool(name="psY", bufs=4, space="PSUM") as psY,
        ):
            c_lhsa = cpool.tile([98, NU * HW], F16)
            c_lhsb = cpool.tile([49, NU * HW], F16)
            nc.sync.dma_start(out=c_lhsa, in_=lhsa.ap())
            nc.sync.dma_start(out=c_lhsb, in_=lhsb.ap())
            x_ap = xst2.ap()
            y_ap = yst.ap()
            for ti, (u, base) in enumerate(sched):
                xt = xpool.tile([98, TILE], F16, tag="xt")
                nc.sync.dma_start(out=xt, in_=x_ap[:, base : base + TILE])
                ps = psY.tile([49, TILE], F32, tag="ps")
                nc.tensor.matmul(
                    ps, c_lhsa[:, u * HW : (u + 1) * HW], xt, start=True, stop=False
                )
                nc.tensor.matmul(
                    ps,
                    c_lhsb[:, u * HW : (u + 1) * HW],
                    xt[0:49, :],
                    start=False,
                    stop=True,
                )
                yt = ypool.tile([49, TILE], F32, tag="yt")
                if ti % 2 == 0:
                    nc.scalar.copy(yt, ps)
                else:
                    nc.vector.tensor_copy(yt, ps)
                nc.sync.dma_start(out=y_ap[:, base : base + TILE], in_=yt)
    nc.compile()
    return nc


# ---------------------------------------------------------------------------
# host orchestration
# ---------------------------------------------------------------------------


def kernel(x: np.ndarray) -> tuple[np.ndarray, np.ndarray]:
    x = np.asarray(x)
    assert x.shape == (B, 1, H, W) and x.dtype == np.float32
    xf = np.ascontiguousarray(x.reshape(B, HW))
    shards = xf.reshape(N_CORES, PER, HW)

    xhi = shards.astype(np.float16)
    xlo = (shards - xhi.astype(np.float32)).astype(np.float16)

    # ---- launch 1 ----
    nc1 = build_decision_kernel()
    in_maps = []
    for c in range(N_CORES):
        xt2 = np.concatenate(
            [np.ascontiguousarray(xhi[c].T), np.ascontiguousarray(xlo[c].T)], axis=0
        )  # [98, PER] hi rows first
        in_maps.append(
            {
                "xt2": xt2,
                "lhsa": L1_LHSA.astype(np.float16),
                "lhsb": L1_LHSB.astype(np.float16),
                "ident": L1_IDENT,
                "iota": L1_IOTA,
            }
        )
    res1 = run_bass_kernel_spmd(
        nc1, in_maps, core_ids=list(range(N_CORES)), trace=_TRACE
    )
    LAST_EXEC_NS["decision"] = res1.exec_time_ns
    k_all = np.stack([decode_mx(res1.results[c]["mx"]) for c in range(N_CORES)])

    theta = VALID_THETAS[k_all.reshape(B)]

    # ---- host glue: class-sorted layout ----
    uid = BIN2UID[k_all]  # [N_CORES, PER]
    counts = np.stack([np.bincount(uid[c], minlength=NU) for c in range(N_CORES)])
    caps = ((counts.max(axis=0) + TILE - 1) // TILE) * TILE
    cap_off = np.concatenate([[0], np.cumsum(caps)])[:NU]
    cap_total = int(caps.sum())

    slots = np.empty((N_CORES, PER), dtype=np.int64)
    for c in range(N_CORES):
        order = np.argsort(uid[c], kind="stable")
        rank = np.empty(PER, dtype=np.int64)
        rank[order] = np.arange(PER)
        cum = np.concatenate([[0], np.cumsum(counts[c])])[:NU]
        slots[c] = cap_off[uid[c]] + (rank - cum[uid[c]])

    # ---- launch 2 ----
    nc2 = build_apply_kernel(caps)
    in_maps2 = []
    for c in range(N_CORES):
        xst2 = np.zeros((98, cap_total), dtype=np.float16)
        xst2[0:49, slots[c]] = xhi[c].T
        xst2[49:98, slots[c]] = xlo[c].T
        in_maps2.append(
            {
                "xst2": xst2,
                "lhsa": L2_LHSA.astype(np.float16),
                "lhsb": L2_LHSB.astype(np.float16),
            }
        )
    res2 = run_bass_kernel_spmd(
        nc2, in_maps2, core_ids=list(range(N_CORES)), trace=_TRACE
    )
    LAST_EXEC_NS["apply"] = res2.exec_time_ns

    out = np.empty((N_CORES, PER, HW), dtype=np.float32)
    for c in range(N_CORES):
        yst = res2.results[c]["yst"]  # [49, cap_total]
        out[c] = yst[:, slots[c]].T

    return out.reshape(B, 1, H, W), theta


# revision 13
# speedup vs baseline: 1.1789x; 1.1789x over previous
"""Trainium2 Bass kernel for nn_FAA (Fourier-argmax alignment).

Per sample (7x7 image): rfft2 magnitudes -> argmax over 27 frequency bins
(weighted by rho) -> rotation angle theta -> bilinear grid-sample rotation.

Device pipeline (2 launches, 8 cores, batch sharded):
  Launch 1 (decision): FFT as matmul (fp16 hi/lo, fp32-accurate), square,
    |F|^2*rho^2 per bin, argmax via mantissa-packed segmented max-reduce.
  Host glue: bin -> unique-angle class, class-sort layout (counting sort).
  Launch 2 (apply): per-class rotation matrices as dense matmuls over the
    class-sorted stream (fp16 hi/lo, fp32-accurate).
"""

import os
import sys

import numpy as np

sys.path.insert(0, "/opt/trn_rl_repo")

import concourse.bacc as bacc
import concourse.tile as tile
from concourse import mybir
from concourse.bass_utils import run_bass_kernel_spmd

F32 = mybir.dt.float32
F16 = mybir.dt.float16
U32 = mybir.dt.uint32

H = W = 7
HW = 49
B = 524288
N_CORES = 8
PER = B // N_CORES  # 65536
GRP = 1024  # samples per launch-1 group (2 subtiles of 512)
N_GRP = PER // GRP  # 64
STAGE_GROUPS = 8  # groups per Mx staging DMA
TILE = 512  # samples per launch-2 matmul tile

LAST_EXEC_NS = {"decision": None, "apply": None}
_TRACE = os.environ.get("FAA_TRACE", "0") == "1"

# ---------------------------------------------------------------------------
# constants (pure math, replicates the reference module exactly)
# ---------------------------------------------------------------------------


def _freq_constants():
    h_shift = np.roll(np.arange(H) - H // 2, H // 2)
    w_shift = np.concatenate([np.arange(W // 2 + 1)[: W // 2], np.array([-(W // 2)])])
    y, xg = np.meshgrid(h_shift, w_shift, indexing="ij")
    rho = np.sqrt((xg**2 + y**2).astype(np.float32))
    theta = np.arctan2(y.astype(np.float32), xg.astype(np.float32))
    theta = (theta + 2.0 * np.pi) % (2.0 * np.pi)
    valid = np.flatnonzero((rho > 1e-8).ravel())
    return valid, theta.ravel()[valid].astype(np.float32), rho.ravel()[valid].astype(
        np.float32
    )


VALID_IDX, VALID_THETAS, VALID_RHOS = _freq_constants()
NBINS = len(VALID_IDX)  # 27


def _fourier_mats():
    C = np.zeros((NBINS, HW), dtype=np.float64)
    S = np.zeros((NBINS, HW), dtype=np.float64)
    for j, flat in enumerate(VALID_IDX):
        u, v = divmod(int(flat), W // 2 + 1)
        for h in range(H):
            for w in range(W):
                ang = 2.0 * np.pi * (u * h + v * w) / 7.0
                C[j, h * W + w] = np.cos(ang) / 7.0
                S[j, h * W + w] = -np.sin(ang) / 7.0
    return C.astype(np.float32), S.astype(np.float32)


CMAT, SMAT = _fourier_mats()

_ys = np.linspace(-1.0, 1.0, H, dtype=np.float32)
_xs = np.linspace(-1.0, 1.0, W, dtype=np.float32)
GY, GX = np.meshgrid(_ys, _xs, indexing="ij")[0], np.meshgrid(_ys, _xs, indexing="ij")[
    1
]


def _rotation_matrix(theta):
    c = np.float32(np.cos(theta))
    s = np.float32(np.sin(theta))
    sx = GX * c - GY * s
    sy = GX * s + GY * c
    ix = np.clip((sx + np.float32(1)) * np.float32(0.5) * (W - 1), 0.0, W - 1).astype(
        np.float32
    )
    iy = np.clip((sy + np.float32(1)) * np.float32(0.5) * (H - 1), 0.0, H - 1).astype(
        np.float32
    )
    ix0f = np.floor(ix)
    iy0f = np.floor(iy)
    wx = (ix - ix0f).astype(np.float32)
    wy = (iy - iy0f).astype(np.float32)
    ix0 = ix0f.astype(np.int64)
    iy0 = iy0f.astype(np.int64)
    ix1 = np.minimum(ix0 + 1, W - 1)
    iy1 = np.minimum(iy0 + 1, H - 1)
    A = np.zeros((HW, HW), dtype=np.float32)
    for r in range(H):
        for cc in range(W):
            p = r * W + cc
            A[p, iy0[r, cc] * W + ix0[r, cc]] += (1 - wy[r, cc]) * (1 - wx[r, cc])
            A[p, iy0[r, cc] * W + ix1[r, cc]] += (1 - wy[r, cc]) * wx[r, cc]
            A[p, iy1[r, cc] * W + ix0[r, cc]] += wy[r, cc] * (1 - wx[r, cc])
            A[p, iy1[r, cc] * W + ix1[r, cc]] += wy[r, cc] * wx[r, cc]
    return A


UNIQ_THETAS, BIN2UID = np.unique(VALID_THETAS, return_inverse=True)
NU = len(UNIQ_THETAS)  # 20
AMATS = np.stack([_rotation_matrix(t) for t in UNIQ_THETAS])  # [NU, 49, 49]


def _split16(a):
    hi = a.astype(np.float16)
    lo = (a.astype(np.float32) - hi.astype(np.float32)).astype(np.float16)
    return hi, lo


def _build_l1_consts():
    # CS64 [49, 64]: col 2j -> rho_j*C_j ; col 2j+1 -> rho_j*S_j (interleaved
    # so Re^2+Im^2 is a segmented pair reduce after transpose)
    cs = np.zeros((HW, 64), dtype=np.float32)
    cs[:, 0 : 2 * NBINS : 2] = (CMAT * VALID_RHOS[:, None]).T
    cs[:, 1 : 2 * NBINS : 2] = (SMAT * VALID_RHOS[:, None]).T
    cs_hi, cs_lo = _split16(cs)
    # term A lhsT [98, 64] = [Clo ; Chi] (matches rhs [Xhi ; Xlo])
    lhsA = np.concatenate([cs_lo, cs_hi], axis=0)
    lhsB = cs_hi  # [49, 64], rhs = Xhi rows
    ident = np.eye(128, dtype=np.float32)
    iota = np.broadcast_to(
        np.tile(np.arange(32, dtype=np.uint32), 16), (128, 16 * 32)
    ).copy()
    return lhsA, lhsB, ident, iota


L1_LHSA, L1_LHSB, L1_IDENT, L1_IOTA = _build_l1_consts()


def _build_l2_consts():
    # per class u: lhsT = [Ahi_u^T ; Ahi_u^T] (K=98 vs rhs [Xhi ; Xlo])
    # => out = Ahi^T.T @ (Xhi + Xlo) = Ahi @ x ; fp16-A error ~2e-4 rel, fine
    at = AMATS.transpose(0, 2, 1).astype(np.float32)  # [NU, 49, 49] lhsT
    hi = at.astype(np.float16)
    lhsA = np.concatenate([hi, hi], axis=1)  # [NU, 98, 49]
    lhsA = np.concatenate(list(lhsA), axis=1)  # [98, NU*49]
    return np.ascontiguousarray(lhsA)


L2_LHSA = _build_l2_consts()

# ---------------------------------------------------------------------------
# launch 1: decision kernel
# ---------------------------------------------------------------------------


SUPER = 2048  # samples per super-group (4 subtiles of 512)
N_SUPER = PER // SUPER  # 32
STAGE_SUPER = 4  # super-groups per Mx staging DMA -> [128, 64] f32


def build_decision_kernel():
    nc = bacc.Bacc("TRN2", target_bir_lowering=False, debug=False, num_devices=N_CORES)
    xt2 = nc.dram_tensor("xt2", [98, PER], F16, kind="ExternalInput")
    lhsa = nc.dram_tensor("lhsa", [98, 64], F16, kind="ExternalInput")
    lhsb = nc.dram_tensor("lhsb", [49, 64], F16, kind="ExternalInput")
    ident = nc.dram_tensor("ident", [128, 128], F32, kind="ExternalInput")
    iota = nc.dram_tensor("iota", [128, 16 * 32], U32, kind="ExternalInput")
    mx = nc.dram_tensor(
        "mx",
        [N_SUPER // STAGE_SUPER, 128, STAGE_SUPER * 16],
        F32,
        kind="ExternalOutput",
    )

    with tile.TileContext(nc) as tc:
        with (
            tc.tile_pool(name="const", bufs=1) as cpool,
            tc.tile_pool(name="xin", bufs=8) as xpool,
            tc.tile_pool(name="sq", bufs=2) as sqpool,
            tc.tile_pool(name="w", bufs=2) as wpool,
            tc.tile_pool(name="stage", bufs=2) as stpool,
            tc.tile_pool(name="psF", bufs=2, space="PSUM") as psF,
            tc.tile_pool(name="psT", bufs=2, space="PSUM") as psT,
        ):
            c_lhsa = cpool.tile([98, 64], F16)
            c_lhsb = cpool.tile([49, 64], F16)
            c_id = cpool.tile([128, 128], F32)
            c_iota = cpool.tile([128, 16 * 32], U32)
            nc.sync.dma_start(out=c_lhsa, in_=lhsa.ap())
            nc.sync.dma_start(out=c_lhsb, in_=lhsb.ap())
            nc.sync.dma_start(out=c_id, in_=ident.ap())
            nc.sync.dma_start(out=c_iota, in_=iota.ap())

            xt2_ap = xt2.ap()
            for gs in range(N_SUPER // STAGE_SUPER):
                stg = stpool.tile([128, STAGE_SUPER * 16], F32, tag="stage")
                for gi in range(STAGE_SUPER):
                    g = gs * STAGE_SUPER + gi
                    base = g * SUPER
                    # 4 subtiles [98, 512] fp16
                    xs = []
                    for s in range(4):
                        xt_ = xpool.tile([98, TILE], F16, tag=f"x{s}", name=f"x{s}")
                        nc.sync.dma_start(
                            out=xt_,
                            in_=xt2_ap[:, base + s * TILE : base + (s + 1) * TILE],
                        )
                        xs.append(xt_)
                    # FFT into one psum [128, 1024]; weight-batched:
                    # 4x term A (lhsa), then 4x term B (lhsb)
                    f8 = psF.tile([128, 2 * TILE], F32, tag="f8")
                    quads = [
                        (f8[0:64, 0:TILE], (0, 0), xs[0]),
                        (f8[64:128, 0:TILE], (0, 64), xs[1]),
                        (f8[0:64, TILE : 2 * TILE], (0, 0), xs[2]),
                        (f8[64:128, TILE : 2 * TILE], (0, 64), xs[3]),
                    ]
                    for out_ap, tp, xt_ in quads:
                        nc.tensor.matmul(
                            out_ap, c_lhsa, xt_, start=True, stop=False, tile_position=tp
                        )
                    for out_ap, tp, xt_ in quads:
                        nc.tensor.matmul(
                            out_ap,
                            c_lhsb,
                            xt_[0:49, :],
                            start=False,
                            stop=True,
                            tile_position=tp,
                        )
                    # square (ACT) -> sbuf [128, 1024]
                    fsq = sqpool.tile([128, 2 * TILE], F32, tag="fsq")
                    nc.scalar.activation(
                        out=fsq, in_=f8, func=mybir.ActivationFunctionType.Square
                    )
                    # PE transpose 8 blocks -> psum FsqT [128, 1024]
                    ft = psT.tile([128, 2 * TILE], F32, tag="ft")
                    for b in range(8):
                        nc.tensor.transpose(
                            ft[:, 128 * b : 128 * (b + 1)],
                            fsq[:, 128 * b : 128 * (b + 1)],
                            c_id,
                        )
                    # W = Re^2 + Im^2 : segmented pair reduce [128, 16, 32]
                    wt = wpool.tile([128, 16, 32], F32, tag="wt")
                    nc.vector.tensor_reduce(
                        out=wt,
                        in_=ft.rearrange("p (s b t) -> p s b t", s=16, b=32),
                        axis=mybir.AxisListType.X,
                        op=mybir.AluOpType.add,
                    )
                    # pack bin id into low 5 mantissa bits
                    wm = wpool.tile([128, 16, 32], F32, tag="wm")
                    nc.vector.tensor_scalar(
                        out=wm.bitcast(U32),
                        in0=wt.bitcast(U32),
                        scalar1=0xFFFFFFE0,
                        scalar2=None,
                        op0=mybir.AluOpType.bitwise_and,
                    )
                    wp = wpool.tile([128, 16, 32], F32, tag="wp")
                    nc.vector.tensor_tensor(
                        out=wp.bitcast(U32),
                        in0=wm.bitcast(U32),
                        in1=c_iota.rearrange("p (s c) -> p s c", c=32),
                        op=mybir.AluOpType.bitwise_or,
                    )
                    # segmented max over bins -> [128, 16]
                    nc.vector.tensor_reduce(
                        out=stg[:, gi * 16 : (gi + 1) * 16],
                        in_=wp,
                        axis=mybir.AxisListType.X,
                        op=mybir.AluOpType.max,
                    )
                nc.sync.dma_start(out=mx.ap()[gs], in_=stg)
    nc.compile()
    return nc


def decode_mx(mx_arr):
    """mx [N_SUPER//SS, 128, SS*16] f32 -> k [PER] int (bin index).

    col = gi*16 + seg ; seg = 2*b + s (b in 0..7, s in 0..1)
    sample = (gs*SS+gi)*2048 + (b//4)*1024 + s*512 + 128*(b%4) + i
    """
    bits = np.minimum(mx_arr.view(np.uint32) & np.uint32(31), NBINS - 1)
    k = np.empty(PER, dtype=np.int64)
    gs_, i_, c_ = np.indices(bits.shape)
    gi_ = c_ // 16
    seg = c_ % 16
    b_ = seg // 2
    s_ = seg % 2
    sample = (
        (gs_ * STAGE_SUPER + gi_) * SUPER
        + (b_ // 4) * 1024
        + s_ * 512
        + 128 * (b_ % 4)
        + i_
    )
    k[sample.ravel()] = bits.ravel()
    return k


# ---------------------------------------------------------------------------
# launch 2: apply kernel (schedule depends on per-class capacities)
# ---------------------------------------------------------------------------


def build_apply_kernel(caps):
    cap_total = int(np.sum(caps))
    nc = bacc.Bacc("TRN2", target_bir_lowering=False, debug=False, num_devices=N_CORES)
    xst2 = nc.dram_tensor("xst2", [98, cap_total], F16, kind="ExternalInput")
    lhsa = nc.dram_tensor("lhsa", [98, NU * HW], F16, kind="ExternalInput")
    yst = nc.dram_tensor("yst", [49, cap_total], F32, kind="ExternalOutput")

    # tile schedule: (class u, slot base)
    sched = []
    off = 0
    for u in range(NU):
        for t in range(int(caps[u]) // TILE):
            sched.append((u, off + t * TILE))
        off += int(caps[u])

    with tile.TileContext(nc) as tc:
        with (
            tc.tile_pool(name="const", bufs=1) as cpool,
            tc.tile_pool(name="xin", bufs=8) as xpool,
            tc.tile_pool(name="yout", bufs=8) as ypool,
            tc.tile_pool(name="psY", bufs=2, space="PSUM") as psY,
        ):
            c_lhsa = cpool.tile([98, NU * HW], F16)
            nc.sync.dma_start(out=c_lhsa, in_=lhsa.ap())
            x_ap = xst2.ap()
            y_ap = yst.ap()
            # batch tiles in quads: loads, 4x matmul (same weights within a
            # class run), copies (alternate ACT/DVE), stores
            for q0 in range(0, len(sched), 4):
                quad = sched[q0 : q0 + 4]
                xts = []
                for qi, (u, base) in enumerate(quad):
                    xt = xpool.tile([98, TILE], F16, tag=f"xt{qi}", name=f"xt{qi}")
                    nc.sync.dma_start(out=xt, in_=x_ap[:, base : base + TILE])
                    xts.append(xt)
                pss = []
                for qi, (u, base) in enumerate(quad):
                    ps = psY.tile([49, TILE], F32, tag=f"ps{qi}", name=f"ps{qi}")
                    nc.tensor.matmul(
                        ps,
                        c_lhsa[:, u * HW : (u + 1) * HW],
                        xts[qi],
                        start=True,
                        stop=True,
                    )
                    pss.append(ps)
                for qi, (u, base) in enumerate(quad):
                    yt = ypool.tile([49, TILE], F32, tag=f"yt{qi}", name=f"yt{qi}")
                    if qi % 2 == 0:
                        nc.scalar.copy(yt, pss[qi])
                    else:
                        nc.vector.tensor_copy(yt, pss[qi])
                    nc.sync.dma_start(out=y_ap[:, base : base + TILE], in_=yt)
    nc.compile()
    return nc


# ---------------------------------------------------------------------------
# host orchestration
# ---------------------------------------------------------------------------


def kernel(x: np.ndarray) -> tuple[np.ndarray, np.ndarray]:
    x = np.asarray(x)
    assert x.shape == (B, 1, H, W) and x.dtype == np.float32
    xf = np.ascontiguousarray(x.reshape(B, HW))
    shards = xf.reshape(N_CORES, PER, HW)

    xhi = shards.astype(np.float16)
    xlo = (shards - xhi.astype(np.float32)).astype(np.float16)

    # ---- launch 1 ----
    nc1 = build_decision_kernel()
    in_maps = []
    for c in range(N_CORES):
        xt2 = np.concatenate(
            [np.ascontiguousarray(xhi[c].T), np.ascontiguousarray(xlo[c].T)], axis=0
        )  # [98, PER] hi rows first
        in_maps.append(
            {
                "xt2": xt2,
                "lhsa": L1_LHSA.astype(np.float16),
                "lhsb": L1_LHSB.astype(np.float16),
                "ident": L1_IDENT,
                "iota": L1_IOTA,
            }
        )
    res1 = run_bass_kernel_spmd(
        nc1, in_maps, core_ids=list(range(N_CORES)), trace=_TRACE
    )
    LAST_EXEC_NS["decision"] = res1.exec_time_ns
    k_all = np.stack([decode_mx(res1.results[c]["mx"]) for c in range(N_CORES)])

    theta = VALID_THETAS[k_all.reshape(B)]

    # ---- host glue: class-sorted layout ----
    uid = BIN2UID[k_all]  # [N_CORES, PER]
    counts = np.stack([np.bincount(uid[c], minlength=NU) for c in range(N_CORES)])
    caps = ((counts.max(axis=0) + TILE - 1) // TILE) * TILE
    cap_off = np.concatenate([[0], np.cumsum(caps)])[:NU]
    cap_total = int(caps.sum())

    slots = np.empty((N_CORES, PER), dtype=np.int64)
    for c in range(N_CORES):
        order = np.argsort(uid[c], kind="stable")
        rank = np.empty(PER, dtype=np.int64)
        rank[order] = np.arange(PER)
        cum = np.concatenate([[0], np.cumsum(counts[c])])[:NU]
        slots[c] = cap_off[uid[c]] + (rank - cum[uid[c]])

    # ---- launch 2 ----
    nc2 = build_apply_kernel(caps)
    in_maps2 = []
    for c in range(N_CORES):
        xst2 = np.zeros((98, cap_total), dtype=np.float16)
        xst2[0:49, slots[c]] = xhi[c].T
        xst2[49:98, slots[c]] = xlo[c].T
        in_maps2.append({"xst2": xst2, "lhsa": L2_LHSA})
    res2 = run_bass_kernel_spmd(
        nc2, in_maps2, core_ids=list(range(N_CORES)), trace=_TRACE
    )
    LAST_EXEC_NS["apply"] = res2.exec_time_ns

    out = np.empty((N_CORES, PER, HW), dtype=np.float32)
    for c in range(N_CORES):
        yst = res2.results[c]["yst"]  # [49, cap_total]
        out[c] = yst[:, slots[c]].T

    return out.reshape(B, 1, H, W), theta


# revision 15
# speedup vs baseline: 1.6966x; 1.4391x over previous
"""Trainium2 Bass kernel for nn_FAA (Fourier-argmax alignment).

Per sample (7x7 image): rfft2 magnitudes -> argmax over 27 frequency bins
(weighted by rho) -> rotation angle theta -> bilinear grid-sample rotation.

Device pipeline (2 launches, 8 cores, batch sharded):
  Launch 1 (decision): FFT as matmul (fp16 hi/lo, fp32-accurate), square,
    |F|^2*rho^2 per bin, argmax via mantissa-packed segmented max-reduce.
  Host glue: bin -> unique-angle class, class-sort layout (counting sort).
  Launch 2 (apply): per-class rotation matrices as dense matmuls over the
    class-sorted stream (fp16 hi/lo, fp32-accurate).
"""

import os
import sys

import numpy as np

sys.path.insert(0, "/opt/trn_rl_repo")

import concourse.bacc as bacc
import concourse.tile as tile
from concourse import mybir
from concourse.bass_utils import run_bass_kernel_spmd

F32 = mybir.dt.float32
F16 = mybir.dt.float16
U32 = mybir.dt.uint32

H = W = 7
HW = 49
B = 524288
N_CORES = 8
PER = B // N_CORES  # 65536
GRP = 1024  # samples per launch-1 group (2 subtiles of 512)
N_GRP = PER // GRP  # 64
STAGE_GROUPS = 8  # groups per Mx staging DMA
TILE = 512  # samples per launch-2 matmul tile

LAST_EXEC_NS = {"decision": None, "apply": None}
_TRACE = os.environ.get("FAA_TRACE", "0") == "1"

# ---------------------------------------------------------------------------
# constants (pure math, replicates the reference module exactly)
# ---------------------------------------------------------------------------


def _freq_constants():
    h_shift = np.roll(np.arange(H) - H // 2, H // 2)
    w_shift = np.concatenate([np.arange(W // 2 + 1)[: W // 2], np.array([-(W // 2)])])
    y, xg = np.meshgrid(h_shift, w_shift, indexing="ij")
    rho = np.sqrt((xg**2 + y**2).astype(np.float32))
    theta = np.arctan2(y.astype(np.float32), xg.astype(np.float32))
    theta = (theta + 2.0 * np.pi) % (2.0 * np.pi)
    valid = np.flatnonzero((rho > 1e-8).ravel())
    return valid, theta.ravel()[valid].astype(np.float32), rho.ravel()[valid].astype(
        np.float32
    )


VALID_IDX, VALID_THETAS, VALID_RHOS = _freq_constants()
NBINS = len(VALID_IDX)  # 27


def _fourier_mats():
    C = np.zeros((NBINS, HW), dtype=np.float64)
    S = np.zeros((NBINS, HW), dtype=np.float64)
    for j, flat in enumerate(VALID_IDX):
        u, v = divmod(int(flat), W // 2 + 1)
        for h in range(H):
            for w in range(W):
                ang = 2.0 * np.pi * (u * h + v * w) / 7.0
                C[j, h * W + w] = np.cos(ang) / 7.0
                S[j, h * W + w] = -np.sin(ang) / 7.0
    return C.astype(np.float32), S.astype(np.float32)


CMAT, SMAT = _fourier_mats()

_ys = np.linspace(-1.0, 1.0, H, dtype=np.float32)
_xs = np.linspace(-1.0, 1.0, W, dtype=np.float32)
GY, GX = np.meshgrid(_ys, _xs, indexing="ij")[0], np.meshgrid(_ys, _xs, indexing="ij")[
    1
]


def _rotation_matrix(theta):
    c = np.float32(np.cos(theta))
    s = np.float32(np.sin(theta))
    sx = GX * c - GY * s
    sy = GX * s + GY * c
    ix = np.clip((sx + np.float32(1)) * np.float32(0.5) * (W - 1), 0.0, W - 1).astype(
        np.float32
    )
    iy = np.clip((sy + np.float32(1)) * np.float32(0.5) * (H - 1), 0.0, H - 1).astype(
        np.float32
    )
    ix0f = np.floor(ix)
    iy0f = np.floor(iy)
    wx = (ix - ix0f).astype(np.float32)
    wy = (iy - iy0f).astype(np.float32)
    ix0 = ix0f.astype(np.int64)
    iy0 = iy0f.astype(np.int64)
    ix1 = np.minimum(ix0 + 1, W - 1)
    iy1 = np.minimum(iy0 + 1, H - 1)
    A = np.zeros((HW, HW), dtype=np.float32)
    for r in range(H):
        for cc in range(W):
            p = r * W + cc
            A[p, iy0[r, cc] * W + ix0[r, cc]] += (1 - wy[r, cc]) * (1 - wx[r, cc])
            A[p, iy0[r, cc] * W + ix1[r, cc]] += (1 - wy[r, cc]) * wx[r, cc]
            A[p, iy1[r, cc] * W + ix0[r, cc]] += wy[r, cc] * (1 - wx[r, cc])
            A[p, iy1[r, cc] * W + ix1[r, cc]] += wy[r, cc] * wx[r, cc]
    return A


UNIQ_THETAS, BIN2UID = np.unique(VALID_THETAS, return_inverse=True)
NU = len(UNIQ_THETAS)  # 20
AMATS = np.stack([_rotation_matrix(t) for t in UNIQ_THETAS])  # [NU, 49, 49]


def _split16(a):
    hi = a.astype(np.float16)
    lo = (a.astype(np.float32) - hi.astype(np.float32)).astype(np.float16)
    return hi, lo


def _build_l1_consts():
    # CS64 [49, 64]: col 2j -> rho_j*C_j ; col 2j+1 -> rho_j*S_j (interleaved
    # so Re^2+Im^2 is a segmented pair reduce after transpose)
    cs = np.zeros((HW, 64), dtype=np.float32)
    cs[:, 0 : 2 * NBINS : 2] = (CMAT * VALID_RHOS[:, None]).T
    cs[:, 1 : 2 * NBINS : 2] = (SMAT * VALID_RHOS[:, None]).T
    cs_hi, cs_lo = _split16(cs)
    # term A lhsT [98, 64] = [Clo ; Chi] (matches rhs [Xhi ; Xlo])
    lhsA = np.concatenate([cs_lo, cs_hi], axis=0)
    lhsB = cs_hi  # [49, 64], rhs = Xhi rows
    ident = np.eye(128, dtype=np.float32)
    iota = np.broadcast_to(
        np.tile(np.arange(32, dtype=np.uint32), 16), (128, 16 * 32)
    ).copy()
    return lhsA, lhsB, ident, iota


L1_LHSA, L1_LHSB, L1_IDENT, L1_IOTA = _build_l1_consts()


def _build_l2_consts():
    # per class u: lhsT = [Ahi_u^T ; Ahi_u^T] (K=98 vs rhs [Xhi ; Xlo])
    # => out = Ahi^T.T @ (Xhi + Xlo) = Ahi @ x ; fp16-A error ~2e-4 rel, fine
    at = AMATS.transpose(0, 2, 1).astype(np.float32)  # [NU, 49, 49] lhsT
    hi = at.astype(np.float16)
    lhsA = np.concatenate([hi, hi], axis=1)  # [NU, 98, 49]
    lhsA = np.concatenate(list(lhsA), axis=1)  # [98, NU*49]
    return np.ascontiguousarray(lhsA)


L2_LHSA = _build_l2_consts()

# ---------------------------------------------------------------------------
# launch 1: decision kernel
# ---------------------------------------------------------------------------


SUPER = 2048  # samples per super-group (4 subtiles of 512)
N_SUPER = PER // SUPER  # 32
STAGE_SUPER = 4  # super-groups per Mx staging DMA -> [128, 64] f32


def build_decision_kernel():
    nc = bacc.Bacc("TRN2", target_bir_lowering=False, debug=False, num_devices=N_CORES)
    xt2 = nc.dram_tensor("xt2", [98, PER], F16, kind="ExternalInput")
    lhsa = nc.dram_tensor("lhsa", [98, 64], F16, kind="ExternalInput")
    lhsb = nc.dram_tensor("lhsb", [49, 64], F16, kind="ExternalInput")
    ident = nc.dram_tensor("ident", [128, 128], F32, kind="ExternalInput")
    iota = nc.dram_tensor("iota", [128, 16 * 32], U32, kind="ExternalInput")
    mx = nc.dram_tensor(
        "mx",
        [N_SUPER // STAGE_SUPER, 128, STAGE_SUPER * 16],
        F32,
        kind="ExternalOutput",
    )

    with tile.TileContext(nc) as tc:
        with (
            tc.tile_pool(name="const", bufs=1) as cpool,
            tc.tile_pool(name="xin", bufs=8) as xpool,
            tc.tile_pool(name="sq", bufs=2) as sqpool,
            tc.tile_pool(name="w", bufs=2) as wpool,
            tc.tile_pool(name="stage", bufs=2) as stpool,
            tc.tile_pool(name="psF", bufs=2, space="PSUM") as psF,
            tc.tile_pool(name="psT", bufs=2, space="PSUM") as psT,
        ):
            c_lhsa = cpool.tile([98, 64], F16)
            c_lhsb = cpool.tile([49, 64], F16)
            c_id = cpool.tile([128, 128], F32)
            c_iota = cpool.tile([128, 16 * 32], U32)
            nc.sync.dma_start(out=c_lhsa, in_=lhsa.ap())
            nc.sync.dma_start(out=c_lhsb, in_=lhsb.ap())
            nc.sync.dma_start(out=c_id, in_=ident.ap())
            nc.sync.dma_start(out=c_iota, in_=iota.ap())

            xt2_ap = xt2.ap()
            for gs in range(N_SUPER // STAGE_SUPER):
                stg = stpool.tile([128, STAGE_SUPER * 16], F32, tag="stage")
                for gi in range(STAGE_SUPER):
                    g = gs * STAGE_SUPER + gi
                    base = g * SUPER
                    # one load for all 4 subtiles [98, 2048] fp16
                    xfull = xpool.tile([98, SUPER], F16, tag="xfull")
                    nc.sync.dma_start(out=xfull, in_=xt2_ap[:, base : base + SUPER])
                    xs = [xfull[:, s * TILE : (s + 1) * TILE] for s in range(4)]
                    # FFT into one psum [128, 1024]; weight-batched:
                    # 4x term A (lhsa), then 4x term B (lhsb)
                    f8 = psF.tile([128, 2 * TILE], F32, tag="f8")
                    quads = [
                        (f8[0:64, 0:TILE], (0, 0), xs[0]),
                        (f8[64:128, 0:TILE], (0, 64), xs[1]),
                        (f8[0:64, TILE : 2 * TILE], (0, 0), xs[2]),
                        (f8[64:128, TILE : 2 * TILE], (0, 64), xs[3]),
                    ]
                    for out_ap, tp, xt_ in quads:
                        nc.tensor.matmul(
                            out_ap, c_lhsa, xt_, start=True, stop=False, tile_position=tp
                        )
                    for out_ap, tp, xt_ in quads:
                        nc.tensor.matmul(
                            out_ap,
                            c_lhsb,
                            xt_[0:49, :],
                            start=False,
                            stop=True,
                            tile_position=tp,
                        )
                    # square (ACT) -> sbuf [128, 1024]
                    fsq = sqpool.tile([128, 2 * TILE], F32, tag="fsq")
                    nc.scalar.activation(
                        out=fsq, in_=f8, func=mybir.ActivationFunctionType.Square
                    )
                    # PE transpose 8 blocks -> psum FsqT [128, 1024]
                    ft = psT.tile([128, 2 * TILE], F32, tag="ft")
                    for b in range(8):
                        nc.tensor.transpose(
                            ft[:, 128 * b : 128 * (b + 1)],
                            fsq[:, 128 * b : 128 * (b + 1)],
                            c_id,
                        )
                    # W = Re^2 + Im^2 : segmented pair reduce [128, 16, 32]
                    wt = wpool.tile([128, 16, 32], F32, tag="wt")
                    nc.vector.tensor_reduce(
                        out=wt,
                        in_=ft.rearrange("p (s b t) -> p s b t", s=16, b=32),
                        axis=mybir.AxisListType.X,
                        op=mybir.AluOpType.add,
                    )
                    # pack bin id into low 5 mantissa bits
                    wm = wpool.tile([128, 16, 32], F32, tag="wm")
                    nc.vector.tensor_scalar(
                        out=wm.bitcast(U32),
                        in0=wt.bitcast(U32),
                        scalar1=0xFFFFFFE0,
                        scalar2=None,
                        op0=mybir.AluOpType.bitwise_and,
                    )
                    wp = wpool.tile([128, 16, 32], F32, tag="wp")
                    nc.vector.tensor_tensor(
                        out=wp.bitcast(U32),
                        in0=wm.bitcast(U32),
                        in1=c_iota.rearrange("p (s c) -> p s c", c=32),
                        op=mybir.AluOpType.bitwise_or,
                    )
                    # segmented max over bins -> [128, 16]
                    nc.vector.tensor_reduce(
                        out=stg[:, gi * 16 : (gi + 1) * 16],
                        in_=wp,
                        axis=mybir.AxisListType.X,
                        op=mybir.AluOpType.max,
                    )
                nc.sync.dma_start(out=mx.ap()[gs], in_=stg)
    nc.compile()
    return nc


def decode_mx(mx_arr):
    """mx [N_SUPER//SS, 128, SS*16] f32 -> k [PER] int (bin index).

    col = gi*16 + seg ; seg = 2*b + s (b in 0..7, s in 0..1)
    sample = (gs*SS+gi)*2048 + (b//4)*1024 + s*512 + 128*(b%4) + i
    """
    bits = np.minimum(mx_arr.view(np.uint32) & np.uint32(31), NBINS - 1)
    k = np.empty(PER, dtype=np.int64)
    gs_, i_, c_ = np.indices(bits.shape)
    gi_ = c_ // 16
    seg = c_ % 16
    b_ = seg // 2
    s_ = seg % 2
    sample = (
        (gs_ * STAGE_SUPER + gi_) * SUPER
        + (b_ // 4) * 1024
        + s_ * 512
        + 128 * (b_ % 4)
        + i_
    )
    k[sample.ravel()] = bits.ravel()
    return k


# ---------------------------------------------------------------------------
# launch 2: apply kernel (schedule depends on per-class capacities)
# ---------------------------------------------------------------------------


def build_apply_kernel(caps):
    cap_total = int(np.sum(caps))
    nc = bacc.Bacc("TRN2", target_bir_lowering=False, debug=False, num_devices=N_CORES)
    xst2 = nc.dram_tensor("xst2", [98, cap_total], F16, kind="ExternalInput")
    lhsa = nc.dram_tensor("lhsa", [98, NU * HW], F16, kind="ExternalInput")
    yst = nc.dram_tensor("yst", [49, cap_total], F32, kind="ExternalOutput")

    # tile schedule: (class u, slot base)
    sched = []
    off = 0
    for u in range(NU):
        for t in range(int(caps[u]) // TILE):
            sched.append((u, off + t * TILE))
        off += int(caps[u])

    with tile.TileContext(nc) as tc:
        with (
            tc.tile_pool(name="const", bufs=1) as cpool,
            tc.tile_pool(name="xin", bufs=3) as xpool,
            tc.tile_pool(name="yout", bufs=3) as ypool,
            tc.tile_pool(name="psY", bufs=2, space="PSUM") as psY,
        ):
            c_lhsa = cpool.tile([98, NU * HW], F16)
            nc.sync.dma_start(out=c_lhsa, in_=lhsa.ap())
            x_ap = xst2.ap()
            y_ap = yst.ap()
            # quads of 4 tiles: 1 load, 4 matmuls into one 4-bank psum,
            # 2 half copies (ACT + DVE), 1 store
            for q0 in range(0, len(sched), 4):
                quad = sched[q0 : q0 + 4]
                nq = len(quad)
                base0 = quad[0][1]
                span = nq * TILE
                xt = xpool.tile([98, 4 * TILE], F16, tag="xt")
                nc.sync.dma_start(
                    out=xt[:, 0:span], in_=x_ap[:, base0 : base0 + span]
                )
                ps = psY.tile([49, 4 * TILE], F32, tag="ps")
                for qi, (u, base) in enumerate(quad):
                    nc.tensor.matmul(
                        ps[:, qi * TILE : (qi + 1) * TILE],
                        c_lhsa[:, u * HW : (u + 1) * HW],
                        xt[:, qi * TILE : (qi + 1) * TILE],
                        start=True,
                        stop=True,
                    )
                yt = ypool.tile([49, 4 * TILE], F32, tag="yt")
                half = (span // 2 + TILE - 1) // TILE * TILE
                nc.scalar.copy(yt[:, 0:half], ps[:, 0:half])
                if span > half:
                    nc.vector.tensor_copy(yt[:, half:span], ps[:, half:span])
                nc.sync.dma_start(
                    out=y_ap[:, base0 : base0 + span], in_=yt[:, 0:span]
                )
    nc.compile()
    return nc


# ---------------------------------------------------------------------------
# host orchestration
# ---------------------------------------------------------------------------


def kernel(x: np.ndarray) -> tuple[np.ndarray, np.ndarray]:
    x = np.asarray(x)
    assert x.shape == (B, 1, H, W) and x.dtype == np.float32
    xf = np.ascontiguousarray(x.reshape(B, HW))
    shards = xf.reshape(N_CORES, PER, HW)

    xhi = shards.astype(np.float16)
    xlo = (shards - xhi.astype(np.float32)).astype(np.float16)

    # ---- launch 1 ----
    nc1 = build_decision_kernel()
    in_maps = []
    for c in range(N_CORES):
        xt2 = np.concatenate(
            [np.ascontiguousarray(xhi[c].T), np.ascontiguousarray(xlo[c].T)], axis=0
        )  # [98, PER] hi rows first
        in_maps.append(
            {
                "xt2": xt2,
                "lhsa": L1_LHSA.astype(np.float16),
                "lhsb": L1_LHSB.astype(np.float16),
                "ident": L1_IDENT,
                "iota": L1_IOTA,
            }
        )
    res1 = run_bass_kernel_spmd(
        nc1, in_maps, core_ids=list(range(N_CORES)), trace=_TRACE
    )
    LAST_EXEC_NS["decision"] = res1.exec_time_ns
    k_all = np.stack([decode_mx(res1.results[c]["mx"]) for c in range(N_CORES)])

    theta = VALID_THETAS[k_all.reshape(B)]

    # ---- host glue: class-sorted layout ----
    uid = BIN2UID[k_all]  # [N_CORES, PER]
    counts = np.stack([np.bincount(uid[c], minlength=NU) for c in range(N_CORES)])
    caps = ((counts.max(axis=0) + TILE - 1) // TILE) * TILE
    cap_off = np.concatenate([[0], np.cumsum(caps)])[:NU]
    cap_total = int(caps.sum())

    slots = np.empty((N_CORES, PER), dtype=np.int64)
    for c in range(N_CORES):
        order = np.argsort(uid[c], kind="stable")
        rank = np.empty(PER, dtype=np.int64)
        rank[order] = np.arange(PER)
        cum = np.concatenate([[0], np.cumsum(counts[c])])[:NU]
        slots[c] = cap_off[uid[c]] + (rank - cum[uid[c]])

    # ---- launch 2 ----
    nc2 = build_apply_kernel(caps)
    in_maps2 = []
    for c in range(N_CORES):
        xst2 = np.zeros((98, cap_total), dtype=np.float16)
        xst2[0:49, slots[c]] = xhi[c].T
        xst2[49:98, slots[c]] = xlo[c].T
        in_maps2.append({"xst2": xst2, "lhsa": L2_LHSA})
    res2 = run_bass_kernel_spmd(
        nc2, in_maps2, core_ids=list(range(N_CORES)), trace=_TRACE
    )
    LAST_EXEC_NS["apply"] = res2.exec_time_ns

    out = np.empty((N_CORES, PER, HW), dtype=np.float32)
    for c in range(N_CORES):
        yst = res2.results[c]["yst"]  # [49, cap_total]
        out[c] = yst[:, slots[c]].T

    return out.reshape(B, 1, H, W), theta
